# revision 73
# baseline (speedup 1.0000x reference)
"""Trainium2 Bass kernel for LogicGatedSNN.

Math:
  w = ternarize(synapse_states)                  # {-1,0,1}
  current = spike_input @ w.T
  spikes[b,o] = (DECAY*vmem[o] + current*(1-refr) >= thr[o])

Implementation (byte-packed fp8, transposed output):
  * Spikes are exactly 0/1, so the host converts them to fp8e4 (a
    lossless layout/dtype change) and the kernel xbar-TRANSPOSES them
    straight from DRAM into SBUF as packed u16 pairs of fp8: su tile
    element [p, m, x] (u16) holds the fp8 pair k = 256*m + 2p {+0,+1}.
    No staging loads at all -- 16 transpose-loads are the whole spike
    feed (and HBM reads 4x less than fp32).
  * Weights: w2 = sign(x-1) + sign(x+1) in {-2,0,2}, produced directly
    in fp8e4 (two ACT sign ops + one DVE add into a 2-block pair tile),
    then transposed per 128-row block j through the xbar as u16 pairs.
    Thresholds are doubled: compare psum(=2*current) >= T2 with
    T2 = 2*(thr - DECAY*vmem), or +-2e30 for refractory neurons.  All
    values exact in fp8e4; PSUM accumulates fp32 -> bit-exact vs the
    fp32 reference.
  * ALL transposes AND the W loads AND the stores ride the single sync
    HWDGE ring (one ring avoids the cost model's cross-queue DMA
    ordering serialization; concurrent xbar transposes from two rings
    also corrupt data on HW).  Threshold loads go on the scalar ring.
  * Matmul (DoubleRowSwInterleave, contraction 256/instr): stationary =
    weight bytes [128, o(step2), i(step1)] per 128-o block j (the HW
    consumes interleaved columns high-to-low, so PSUM partition rows
    are o-reversed; the host un-reverses), moving = spike bytes
    [128, i, b] with N=512 batch columns.  A (j, g) group of 16 MMs
    accumulates into one half of a 2-bank psum tile [128, 1024].
  * The host supplies thr/vmem/refrac with each 128-block REVERSED (to
    match the psum row order), so T2 needs no on-device reversal: K=1
    matmuls with ones turn it into a per-partition column t2sc[:, j].
    The (j, g-pair) epilogue is ONE DVE tensor_scalar is_ge over the
    whole 2-bank psum tile -> fp8 0/1 into a per-j-pair [128, 2, bs]
    output tile; one store per j row-block (fewer epilogue/store ops =
    shorter semaphore-cadence tail).
  * Scheduling: a greedy list scheduler simulates the cost-model
    resources (exclusive DMA-engine pool, serial ACT/DVE/PE, ~1us sem
    latency per dependency hop) and instructions are emitted in
    simulated start order so every FIFO queue sees work in true
    readiness order (mis-ordered queues head-of-line block).

Sharding: 8 cores = 2 (batch) x 4 (out_features): per core
  spike [2048, 4096], syn [1024, 4096], out.T [1024, 2048].
"""

import sys

if "/opt/trn_rl_repo" not in sys.path:
    sys.path.insert(0, "/opt/trn_rl_repo")

import numpy as np

B, IN, OUT = 4096, 4096, 4096
SCHED = dict(LAT=1.0, WLB=0.0, SLB=0.0, GRP=2.4, SGN=7.2, ADD=3.3,
             EPI=0.66, WT=1.79, SPT=1.79, WL=5.83, SL=5.83, PSP=6, SPB=0.0,
             JSEED=0, JAMP=0.0)
GB, GO = 2, 4  # core grid: batch x out_features
DECAY = 0.8
_TENSORS = {}


def build_core_program(nc, tc, bs, os_, in_, instance=0):
    import concourse.mybir as mybir
    from concourse.bass import ts

    FP32 = mybir.dt.float32
    BF16 = mybir.dt.bfloat16
    FP8 = mybir.dt.float8e4
    Op = mybir.AluOpType
    Act = mybir.ActivationFunctionType
    DRS = mybir.MatmulPerfMode.DoubleRowSwInterleave

    spike = nc.dram_tensor("spike", [bs, in_], FP8, kind="ExternalInput")
    syn = nc.dram_tensor("syn", [os_, in_], FP32, kind="ExternalInput")
    thr = nc.dram_tensor("thr", [1, os_], FP32, kind="ExternalInput")
    vmem = nc.dram_tensor("vmem", [1, os_], FP32, kind="ExternalInput")
    refrac = nc.dram_tensor("refrac", [1, os_], FP32, kind="ExternalInput")
    outT = nc.dram_tensor("spikesT", [os_, bs], FP8, kind="ExternalOutput")
    _TENSORS.update(
        spike=spike, syn=syn, thr=thr, vmem=vmem, refrac=refrac, out=outT
    )

    KC16 = in_ // 256  # u16-pair contraction chunks (256 k each)
    NGB = bs // 512  # batch groups (512 b each)
    NJ = os_ // 128  # weight row blocks
    NP = NJ // 2  # weight block pairs

    with (
        tc.tile_pool(name="misc", bufs=1) as misc,
        tc.tile_pool(name="wst", bufs=3) as wst,
        tc.tile_pool(name="wsign", bufs=3) as wsign,
        tc.tile_pool(name="wtern", bufs=2) as wtern,
        tc.tile_pool(name="wf", bufs=1) as wf,
        tc.tile_pool(name="sfp", bufs=1) as sfp,
        tc.tile_pool(name="outp", bufs=2) as outp,
        tc.tile_pool(name="psp", bufs=3, space="PSUM") as psp,
        tc.tile_pool(name="psb", bufs=1, space="PSUM") as psb,
    ):
        # ---- threshold tiles
        a = misc.tile([1, os_], FP32, tag="a")
        b = misc.tile([1, os_], FP32, tag="b")
        r = misc.tile([1, os_], FP32, tag="r")
        nc.scalar.dma_start(b[:], thr[:, :])
        nc.scalar.dma_start(a[:], vmem[:, :])
        nc.scalar.dma_start(r[:], refrac[:, :])
        ones = misc.tile([1, 1], FP32, tag="ones")
        nc.vector.memset(ones[:], 1.0)
        bneg = misc.tile([128, 1], FP32, tag="bneg")
        bpos = misc.tile([128, 1], FP32, tag="bpos")
        nc.vector.memset(bneg[:], -1.0)
        nc.vector.memset(bpos[:], 1.0)
        t2sc = misc.tile([128, NJ], FP32, tag="t2sc")

        # T2 = -2*c0 + refr * (big + 2*c0), c0 = DECAY*vmem - thr,
        # big = (c0>=0) ? -2e30 : +2e30   (always/never spike when refr.)
        thrmath_ops = [
            lambda: nc.vector.tensor_scalar(a[:], a[:], DECAY, None, Op.mult),
            lambda: nc.vector.tensor_tensor(a[:], a[:], b[:], Op.subtract),
            lambda: nc.vector.tensor_scalar(b[:], a[:], 0.0, None, Op.is_ge),
            lambda: nc.vector.tensor_scalar(
                b[:], b[:], -4e30, 2e30, Op.mult, Op.add
            ),
            lambda: nc.vector.tensor_scalar(r[:], r[:], 0.0, None, Op.is_gt),
            lambda: nc.vector.tensor_tensor(b[:], b[:], a[:], Op.add),
            lambda: nc.vector.tensor_tensor(b[:], b[:], a[:], Op.add),
            lambda: nc.vector.tensor_tensor(b[:], b[:], r[:], Op.mult),
            lambda: nc.vector.tensor_scalar(r[:], a[:], -2.0, None, Op.mult),
            lambda: nc.vector.tensor_tensor(r[:], r[:], b[:], Op.add),  # t2
        ]

        # the host supplies thr/vmem/refrac with each 128-block reversed,
        # so r holds T2 already in psum-row (o-reversed) order
        pb_tiles = {}

        def mk_bias_mm(j):
            if "pb" not in pb_tiles:
                pb_tiles["pb"] = psb.tile([128, NJ], FP32, tag="pb", name="pb")
            nc.tensor.matmul(
                pb_tiles["pb"][:, j : j + 1], r[:, ts(j, 128)], ones[:],
                start=True, stop=True
            )

        def mk_t2sc_copy():
            nc.vector.tensor_copy(t2sc[:, :], pb_tiles["pb"][:])

        # ---- feed tiles
        # Wp2[p, P, h, m, c] (u16) = w2 of j=2P+h: pair k = 256m+2p of
        # o-row (2P+h)*128 + c
        Wp2 = wf.tile([128, NP, 2, KC16, 128], BF16, tag="Wp", name="Wp")
        su4 = [
            sfp.tile([128, KC16, 512], BF16, tag=f"su{g}", name=f"su{g}")
            for g in range(NGB)
        ]

        st_tiles, w2p_tiles = {}, {}
        wp_tiles = {}
        sgn_tiles = {}
        ps_tiles = {}
        ob_tiles = {}

        def mk_wload(j):
            st = wst.tile([128, in_], FP32, tag="st", name="st")
            st_tiles[j] = st
            nc.sync.dma_start(st[:], syn[ts(j, 128), :])

        def mk_signs(j):
            st = st_tiles[j]
            s1 = wsign.tile([128, in_], FP8, tag="s1", name="s1")
            s2 = wsign.tile([128, in_], FP8, tag="s2", name="s2")
            sgn_tiles[j] = (s1, s2)
            nc.scalar.activation(s1[:], st[:], Act.Sign, bias=bneg[:])
            nc.scalar.activation(s2[:], st[:], Act.Sign, bias=bpos[:])

        def mk_add(j):
            P, h = divmod(j, 2)
            if P not in w2p_tiles:
                w2p_tiles[P] = wtern.tile(
                    [128, 2, in_], FP8, tag="w2", name="w2"
                )
            s1, s2 = sgn_tiles[j]
            nc.vector.tensor_tensor(
                w2p_tiles[P][:, h, :], s1[:], s2[:], Op.add
            )

        def mk_wtrans(j):
            P, h = divmod(j, 2)
            nc.sync.dma_start_transpose(
                Wp2[:, P, h, :, :], w2p_tiles[P][:, h, :].bitcast(BF16)
            )

        def mk_strans(g, t):
            # transpose-load straight from DRAM (spike is already fp8)
            src = spike[ts(4 * g + t, 128), :]
            nc.sync.dma_start_transpose(
                su4[g][:, :, ts(t, 128)], src.bitcast(BF16)
            )

        def mk_group(j, g):
            P, h = divmod(j, 2)
            if P not in ob_tiles:
                ob_tiles[P] = outp.tile(
                    [128, 2, bs], FP8, tag="ob", name="ob"
                )
            gp, gh = divmod(g, 2)
            if (j, gp) not in ps_tiles:
                ps_tiles[(j, gp)] = psp.tile(
                    [128, 1024], FP32, tag="ps", name="ps"
                )
            ps = ps_tiles[(j, gp)][:, ts(gh, 512)]
            for m in range(KC16):
                lhsT = (
                    Wp2[:, P, h, m, :]
                    .bitcast(FP8)
                    .rearrange("p (o i) -> p o i", i=2)
                )
                rhs = (
                    su4[g][:, m, :]
                    .bitcast(FP8)
                    .rearrange("p (b i) -> p i b", i=2)
                )
                nc.tensor.matmul(
                    ps, lhsT, rhs,
                    start=(m == 0), stop=(m == KC16 - 1), perf_mode=DRS,
                )

        def mk_epi(j, gp):
            P, h = divmod(j, 2)
            nc.vector.tensor_scalar(
                ob_tiles[P][:, h, ts(gp, 1024)], ps_tiles[(j, gp)][:],
                t2sc[:, j : j + 1], None, Op.is_ge,
            )

        def mk_store(j):
            P, h = divmod(j, 2)
            nc.sync.dma_start(outT[ts(j, 128), :], ob_tiles[P][:, h, :])

        # ------------------------------------------------------------------
        # Greedy list scheduler over the cost-model resources.
        LAT = SCHED["LAT"]  # sem-prop latency per dependency hop
        import random as _random
        _rng = _random.Random(SCHED["JSEED"])
        JAMP = SCHED["JAMP"]
        items = {}
        order = []

        def add(name, res, dur, deps, fn, bias=0.0, qdeps=()):
            items[name] = dict(
                res=res, dur=dur, deps=[d for d in deps if d is not None],
                fn=fn, prio=len(order),
                bias=bias + (_rng.uniform(0.0, JAMP) if JAMP else 0.0),
                qdeps=[d for d in qdeps if d is not None],
            )
            order.append(name)

        add("thrload", "DMA", 0.5, [], None)  # 3 tiny loads, emitted inline
        for i, op in enumerate(thrmath_ops):
            add(f"tm{i}", "DVE", 1.2,
                ["thrload" if i == 0 else f"tm{i-1}"], op, bias=-6.0)
        last_tm = f"tm{len(thrmath_ops)-1}"
        for j in range(NJ):
            add(f"biasmm{j}", "PE", 0.05, [last_tm],
                lambda j=j: mk_bias_mm(j))
        add("t2c", "DVE", 0.1, [f"biasmm{j}" for j in range(NJ)],
            lambda: mk_t2sc_copy())

        for j in range(NJ):
            dep = f"sign{j-3}" if j >= 3 else None
            add(f"wload{j}", "DMA", SCHED["WL"], [dep],
                lambda j=j: mk_wload(j), bias=SCHED["WLB"])

        for j in range(NJ):
            add(f"sign{j}", "ACT", SCHED["SGN"],
                [f"wload{j}", f"add{j-3}" if j >= 3 else None],
                lambda j=j: mk_signs(j))
            # wtern ring=1: pair P's adds wait pair P-1's transposes
            P = j // 2
            war = (
                [f"wT{2*P-4}", f"wT{2*P-3}"] if P >= 2 else []
            )
            add(f"add{j}", "DVE", SCHED["ADD"], [f"sign{j}"] + war,
                lambda j=j: mk_add(j))
        for j in range(NJ):
            add(f"wT{j}", "DMA", SCHED["WT"], [f"add{j}"],
                lambda j=j: mk_wtrans(j))
        for g in range(NGB):
            for t in range(4):
                add(f"spT{g}_{t}", "DMA", SCHED["SPT"], [],
                    lambda g=g, t=t: mk_strans(g, t), bias=SCHED["SPB"])

        gpseq = [(j, gp) for j in range(NJ) for gp in range(2)]
        for k, (j, g) in enumerate(
            (j, g) for j in range(NJ) for g in range(NGB)
        ):
            gp = g // 2
            war = []
            kp = gpseq.index((j, gp))
            if kp >= 3:
                war.append("epi%d_%d" % gpseq[kp - 3])  # psum ring=3
            if j >= 4:
                war.append(f"store{j-4}")  # ob pair ring=2
                war.append(f"store{j-3}")
            add(f"grp{j}_{g}", "PE", SCHED["GRP"],
                [f"wT{j}"] + [f"spT{g}_{t}" for t in range(4)] + war,
                lambda j=j, g=g: mk_group(j, g))
        for j in range(NJ):
            for gp in range(2):
                add(f"epi{j}_{gp}", "DVE", 1.35,
                    [f"grp{j}_{2*gp}", f"grp{j}_{2*gp+1}", "t2c"],
                    lambda j=j, gp=gp: mk_epi(j, gp))
        for j in range(NJ):
            add(f"store{j}", "DMA", 0.73,
                [f"epi{j}_{gp}" for gp in range(2)],
                lambda j=j: mk_store(j))

        # greedy simulation: start the item with the earliest feasible
        # start time (ties: creation order)
        done_at = {}
        res_free = {"DMA": 0.0, "ACT": 0.0, "DVE": 0.0, "PE": 0.0,
                    "POOL": 0.0}
        sched = []
        pending = set(items)
        while pending:
            best = None
            for name in pending:
                it = items[name]
                if any(d in pending for d in it["deps"]):
                    continue
                if any(d in pending for d in it["qdeps"]):
                    continue
                est = max(
                    [res_free[it["res"]]]
                    + [done_at[d] + LAT for d in it["deps"]]
                    + [done_at[d] for d in it["qdeps"]]
                )
                key = (est - it["bias"], it["prio"])
                if best is None or key < best[0]:
                    best = (key, name)
            (keyest, _), name = best
            it = items[name]
            est = keyest + it["bias"]
            end = est + it["dur"]
            res_free[it["res"]] = end
            done_at[name] = end
            pending.discard(name)
            sched.append((est, it["prio"], name))

        sched.sort()
        for _, _, name in sched:
            fn = items[name]["fn"]
            if fn is not None:
                fn()
    return outT


def make_nc(bs=B // GB, os_=OUT // GO, in_=IN):
    from concourse import bacc
    from concourse.tile import TileContext

    nc = bacc.Bacc(trn_type="TRN2")
    with TileContext(nc) as tc:
        build_core_program(nc, tc, bs, os_, in_)
    nc.compile()
    return nc


_NC_CACHE = {}


def kernel(
    spike_input,
    synapse_states,
    membrane_potential,
    adaptive_threshold,
    refractory_count,
    _return_results=False,
):
    from concourse.bass_utils import run_bass_kernel_spmd

    import concourse.mybir as mybir

    fp8 = mybir.dt.np(mybir.dt.float8e4)
    # spikes are exactly 0/1 -> fp8 cast is lossless layout conversion
    spike_input = np.ascontiguousarray(np.asarray(spike_input).astype(fp8))
    synapse_states = np.ascontiguousarray(np.asarray(synapse_states, dtype=np.float32))
    membrane_potential = np.asarray(membrane_potential, dtype=np.float32)
    adaptive_threshold = np.asarray(adaptive_threshold, dtype=np.float32)
    refractory_count = np.asarray(refractory_count, dtype=np.float32)

    bs, os_ = B // GB, OUT // GO
    if "nc" not in _NC_CACHE:
        _NC_CACHE["nc"] = make_nc(bs, os_, IN)
    nc = _NC_CACHE["nc"]

    def brev(x):
        # reverse each 128-row block (matches the psum row order)
        return np.ascontiguousarray(
            x.reshape(-1, 128)[:, ::-1].reshape(1, -1)
        )

    in_maps = []
    for c in range(GB * GO):
        bi, oj = divmod(c, GO)
        osl = slice(oj * os_, (oj + 1) * os_)
        in_maps.append(
            {
                "spike": spike_input[bi * bs : (bi + 1) * bs],
                "syn": np.ascontiguousarray(synapse_states[osl]),
                "thr": brev(adaptive_threshold[osl]),
                "vmem": brev(membrane_potential[osl]),
                "refrac": brev(refractory_count[osl]),
            }
        )

    res = run_bass_kernel_spmd(nc, in_maps, core_ids=list(range(GB * GO)))

    full = np.empty((B, OUT), dtype=np.float32)
    for c in range(GB * GO):
        bi, oj = divmod(c, GO)
        # outT rows are o-reversed within each 128-row block; un-reverse,
        # then transpose [os_, bs] -> [bs, os_]
        blkT = res.results[c]["spikesT"].astype(np.float32)
        blkT = blkT.reshape(os_ // 128, 128, bs)[:, ::-1, :].reshape(os_, bs)
        full[bi * bs : (bi + 1) * bs, oj * os_ : (oj + 1) * os_] = blkT.T
    if _return_results:
        return full, res
    return full
